# revision 8
# baseline (speedup 1.0000x reference)
"""Trainium2 Bass kernel for the CfC cell (nn_CfCCell), data-parallel on 8 cores.

Math (per row):
    ff1 = gelu(x_cat @ W_ff1.T + b_ff1)          x_cat = [x, hx]
    ff2 = gelu(ff1 @ W_ff2.T + b_ff2)
    t   = sigmoid(ff2 @ (W_ta+W_tb).T + b_ta+b_tb)      (TS == 1.0)
    ic  = gelu(x @ W_in.T + b_in + input_b)
    rc  = gelu(hx @ W_r.T + r_b)
    out = hx + t * (ic + rc - hx)

Per-core design (batch sharded 8 ways, megatiles of R rows):
  * Activations feature-major ([H, batch]); x/hx transposed on the PE.
  * ff1/ff2/tab matmuls run fp8-e4m3 DoubleRow (weights host-scaled x64,
    un-scaled via the activation instruction's free `scale`); ic/rc stay
    bf16 because their outputs feed the combine directly and fp8 there
    costs ~1e-2 of relative error.
  * One shared PSUM ring (2 slots x 4 banks) serves both matmul z-tiles
    ([128, 2048] f32 -> single N=2048 activation instruction) and the
    transpose staging tiles ([128, 2048] bf16) - ScalarE instruction
    count is the bottleneck, so big ACTs matter.
  * Combine is 3 DVE ops (s=ic+rc; d=s-hxT; f=(u+1)*d via
    scalar_tensor_tensor), and the final "*0.5 + hx" rides the
    transpose-back PSUM->SBUF copy as another scalar_tensor_tensor.
  * DMA uses a contiguous-row mapping ("(p n) f"): partition p holds
    rows p*nb..p*nb+nb-1 of the megatile, so each descriptor moves a
    multi-KB contiguous DRAM run. The batch permutation is consistent
    end-to-end so results land in the right rows.
  * sigmoid(z) = 0.5*tanh(z/2)+0.5 keeps every ScalarE op in the single
    "gelu_and_others" table set (no table reloads).
"""

from contextlib import ExitStack

import ml_dtypes
import numpy as np

import concourse.bacc as bacc
import concourse.bass as bass
import concourse.mybir as mybir
import concourse.tile as tile
from concourse import masks
from concourse.bass_utils import run_bass_kernel_spmd

AF = mybir.ActivationFunctionType
ALU = mybir.AluOpType
DR = mybir.MatmulPerfMode.DoubleRow
BF16 = mybir.dt.bfloat16
FP8 = mybir.dt.float8e4
F32 = mybir.dt.float32
NP_BF16 = ml_dtypes.bfloat16
NP_FP8 = ml_dtypes.float8_e4m3

B, I, H = 131072, 128, 256
N_CORES = 8
B_CORE = B // N_CORES  # 16384
R = 2048               # megatile rows

WS = 64.0              # fp8 weight scale (undone in ACT scale)

# bf16 weight chunks [128, 128]: ic (k=0), rc (k=0,1), each x2 m-blocks
BW_COL = {("ic", 0, 0): 0, ("ic", 0, 1): 1,
          ("rc", 0, 0): 2, ("rc", 0, 1): 3, ("rc", 1, 0): 4, ("rc", 1, 1): 5}
N_BW = 6
# fp8 DoubleRow chunks [128, 2, 128]: ff1 2 pairs x2m, ff2 1x2, tab 1x2
W8_COL = {("ff1", 0, 0): 0, ("ff1", 1, 0): 1, ("ff1", 0, 1): 2, ("ff1", 1, 1): 3,
          ("ff2", 0, 0): 4, ("ff2", 0, 1): 5,
          ("tab", 0, 0): 6, ("tab", 0, 1): 7}
N_W8 = 8
LAYERS = ("ff1", "ff2", "tab", "ic", "rc")
BIAS_COL = {(_l, _m): 2 * _i + _m for _i, _l in enumerate(LAYERS) for _m in range(2)}


def build_nc(b_core: int = B_CORE, r: int = R) -> bass.Bass:
    assert b_core % r == 0 and r % 1024 == 0

    nc = bacc.Bacc("TRN2")
    x_d = nc.dram_tensor("x", [b_core, I], F32, kind="ExternalInput")
    hx_d = nc.dram_tensor("hx", [b_core, H], F32, kind="ExternalInput")
    w_d = nc.dram_tensor("wstack", [N_BW, 128, 128], BF16, kind="ExternalInput")
    w8_d = nc.dram_tensor("w8stack", [N_W8, 2, 128, 128], FP8, kind="ExternalInput")
    b_d = nc.dram_tensor("bstack", [128, 10], F32, kind="ExternalInput")
    out_d = nc.dram_tensor("out", [b_core, H], F32, kind="ExternalOutput")

    with tile.TileContext(nc) as tc, ExitStack() as ctx:
        const = ctx.enter_context(tc.tile_pool(name="const", bufs=1))
        w_sb = const.tile([128, N_BW * 128], BF16)
        nc.sync.dma_start(
            w_sb[:].rearrange("p (c f) -> p c f", c=N_BW),
            w_d[:].rearrange("c p f -> p c f"))
        w8_sb = const.tile([128, N_W8 * 2 * 128], FP8)
        nc.sync.dma_start(
            w8_sb[:].rearrange("p (c k f) -> p c k f", c=N_W8, k=2),
            w8_d[:].rearrange("c k p f -> p c k f"))
        b_sb = const.tile([128, 10], F32)
        nc.sync.dma_start(b_sb[:], b_d[:])
        ident = const.tile([128, 128], BF16)
        masks.make_identity(nc, ident[:])

        io = ctx.enter_context(tc.tile_pool(name="io", bufs=2))
        acts = ctx.enter_context(tc.tile_pool(name="acts", bufs=2))
        tmp = ctx.enter_context(tc.tile_pool(name="tmp", bufs=2))
        # one PSUM ring: slots sized for [128, 2048] f32 (4 banks), used both
        # for matmul z-tiles (f32) and transpose staging (bf16)
        ps = ctx.enter_context(tc.tile_pool(name="ps", bufs=2, space="PSUM"))

        # HAM warm-up: dummy PE work while the first loads land
        warm = ps.tile([128, 1024], BF16, tag="ring")
        for i in range(32):
            nc.tensor.transpose(
                warm[:, (i % 8) * 128:(i % 8 + 1) * 128], ident[:], ident[:])

        xd, hxd, outd = x_d[:], hx_d[:], out_d[:]

        def bw(layer, k, m):
            ci = BW_COL[(layer, k, m)]
            return w_sb[:, ci * 128:(ci + 1) * 128]

        def w8(layer, kp, m):
            ci = W8_COL[(layer, kp, m)]
            return w8_sb[:].rearrange(
                "p (c k f) -> p c k f", c=N_W8, k=2)[:, ci]

        def stage_a(r0, rt, first):
            nbt = rt // 128
            # contiguous-row loads: partition p <- rows r0 + p*nbt .. +nbt-1
            x_nat = io.tile([128, nbt * I], BF16, tag="x_nat")
            hx_nat = io.tile([128, nbt * H], BF16, tag="hx_nat")
            x_src = xd[r0:r0 + rt].rearrange("(p n) f -> p n f", p=128)
            hx_src = hxd[r0:r0 + rt].rearrange("(p n) f -> p n f", p=128)
            ng = 4 if first else 2
            g_nb = nbt // ng
            for g in range(ng):
                ns = slice(g * g_nb, (g + 1) * g_nb)
                nc.gpsimd.dma_start(
                    x_nat[:].rearrange("p (n f) -> p n f", n=nbt)[:, ns],
                    x_src[:, ns])
                nc.gpsimd.dma_start(
                    hx_nat[:].rearrange("p (n f) -> p n f", n=nbt)[:, ns],
                    hx_src[:, ns])

            # transpose to feature-major; copies feed bf16 (ic/rc + combine)
            # and fp8 (ff1 moving, slices [x|h0|h1|x]) activation tiles
            xh_T = acts.tile([128, 3 * r], BF16, tag="xh_T")
            xh8_T = acts.tile([128, 4 * r], FP8, tag="xh8_T")
            npack = 2 if first else 1   # tp tiles per input chunk
            p_nb = nbt // npack

            def tpose(src_tile, src_f, src_c, dsts):
                for q in range(npack):
                    tp = ps.tile([128, p_nb * 128], BF16, tag="ring")
                    for i2 in range(p_nb):
                        n = q * p_nb + i2
                        nc.tensor.transpose(
                            tp[:, i2 * 128:(i2 + 1) * 128],
                            src_tile[:, n * src_f + src_c * 128:
                                     n * src_f + (src_c + 1) * 128],
                            ident[:])
                    cs = q * p_nb * 128
                    for dt_, sl in dsts:
                        nc.vector.tensor_copy(
                            dt_[:, sl * r + cs:sl * r + cs + p_nb * 128],
                            tp[:, 0:p_nb * 128])

            tpose(x_nat, I, 0, [(xh_T, 0), (xh8_T, 0), (xh8_T, 3)])
            tpose(hx_nat, H, 0, [(xh_T, 1), (xh8_T, 1)])
            tpose(hx_nat, H, 1, [(xh_T, 2), (xh8_T, 2)])

            def unit_fp8(layer, n_pairs, moving, func, scale, m, out_t, osl):
                mm = ps.tile([128, rt], F32, tag="ring")
                for kp in range(n_pairs):
                    for j in range(rt // 512):
                        sl = slice(j * 512, (j + 1) * 512)
                        nc.tensor.matmul(
                            mm[:, sl], w8(layer, kp, m),
                            moving[:, kp * 2:(kp + 1) * 2, sl],
                            start=(kp == 0), stop=(kp == n_pairs - 1),
                            perf_mode=DR)
                col = BIAS_COL[(layer, m)]
                nc.scalar.activation(
                    out_t[:, osl], mm[:], func,
                    bias=b_sb[:, col:col + 1], scale=scale)

            def unit_bf16(layer, ks, m, out_t, osl):
                mm = ps.tile([128, rt], F32, tag="ring")
                for ki, k in enumerate(ks):
                    c = k if layer == "ic" else 1 + k
                    for j in range(rt // 512):
                        sl = slice(j * 512, (j + 1) * 512)
                        nc.tensor.matmul(
                            mm[:, sl], bw(layer, k, m),
                            xh_T[:, c * r + j * 512:c * r + (j + 1) * 512],
                            start=(ki == 0), stop=(ki == len(ks) - 1))
                col = BIAS_COL[(layer, m)]
                nc.scalar.activation(
                    out_t[:, osl], mm[:], AF.Gelu,
                    bias=b_sb[:, col:col + 1], scale=1.0)

            def mov4(tile_, nsl):
                # [128, nsl, rt] view over a [128, nsl*r] tile (slice
                # stride r, first rt cols of each slice used)
                return tile_[:].rearrange("p (c b) -> p c b", c=nsl)[:, :, 0:rt]

            xh8_m = mov4(xh8_T, 4)

            ff1 = acts.tile([128, 2 * r], FP8, tag="ff1")
            ff1_m = mov4(ff1, 2)
            ic = acts.tile([128, 2 * r], BF16, tag="ic")
            rc = acts.tile([128, 2 * r], BF16, tag="rc")
            ff2 = acts.tile([128, 2 * r], FP8, tag="ff2")
            ff2_m = mov4(ff2, 2)
            u = acts.tile([128, 2 * r], BF16, tag="u")

            for m in range(2):
                unit_fp8("ff1", 2, xh8_m, AF.Gelu, 1.0 / WS, m,
                         ff1, slice(m * r, m * r + rt))
            for m in range(2):
                unit_bf16("ic", [0], m, ic, slice(m * r, m * r + rt))
            for m in range(2):
                unit_bf16("rc", [0, 1], m, rc, slice(m * r, m * r + rt))
            for m in range(2):
                unit_fp8("ff2", 1, ff1_m, AF.Gelu, 1.0 / WS, m,
                         ff2, slice(m * r, m * r + rt))
            for m in range(2):
                unit_fp8("tab", 1, ff2_m, AF.Tanh, 0.5 / WS, m,
                         u, slice(m * r, m * r + rt))
            return {"r0": r0, "rt": rt, "xh_T": xh_T, "u": u, "ic": ic,
                    "rc": rc, "hx_nat": hx_nat}

        def stage_b(st):
            r0, rt = st["r0"], st["rt"]
            xh_T, u, ic, rc, hx_nat = (st["xh_T"], st["u"], st["ic"],
                                       st["rc"], st["hx_nat"])
            nbt = rt // 128
            # f = (u+1)*(ic+rc-hx)  (= 2*t*(ic+rc-hx)), all bf16 on DVE
            f = acts.tile([128, 2 * r], BF16, tag="f")
            for m in range(2):
                msl = slice(m * r, m * r + rt)
                s = tmp.tile([128, r], BF16, tag="s")
                nc.vector.tensor_add(s[:, 0:rt], ic[:, msl], rc[:, msl])
                d = tmp.tile([128, r], BF16, tag="d")
                nc.vector.tensor_sub(
                    d[:, 0:rt], s[:, 0:rt],
                    xh_T[:, (1 + m) * r:(1 + m) * r + rt])
                nc.vector.scalar_tensor_tensor(
                    f[:, msl], u[:, msl], 1.0, d[:, 0:rt], ALU.add, ALU.mult)

            # transpose back in n-pairs (m0,m1), fuse "*0.5 + hx" into the
            # PSUM->SBUF move, store with the same contiguous-row mapping
            out_nat = io.tile([128, nbt * H], BF16, tag="out_nat")
            out_dst = outd[r0:r0 + rt].rearrange("(p n) f -> p n f", p=128)
            nq = 2
            q_nb = nbt // nq
            for q in range(nq):
                tp = ps.tile([128, q_nb * 256], BF16, tag="ring")
                for i2 in range(q_nb):
                    n = q * q_nb + i2
                    for m in range(2):
                        nc.tensor.transpose(
                            tp[:, i2 * 256 + m * 128:i2 * 256 + (m + 1) * 128],
                            f[:, m * r + n * 128:m * r + (n + 1) * 128],
                            ident[:])
                csl = slice(q * q_nb * 256, (q + 1) * q_nb * 256)
                nc.vector.scalar_tensor_tensor(
                    out_nat[:, csl], tp[:], 0.5, hx_nat[:, csl],
                    ALU.mult, ALU.add)
                nc.gpsimd.dma_start(
                    out_dst[:, q * q_nb:(q + 1) * q_nb],
                    out_nat[:, csl].rearrange("p (n f) -> p n f", n=q_nb))

        if b_core > 2 * r:
            sizes = [1024] + [r] * ((b_core - 2048) // r) + [1024]
        else:
            sizes = [r] * (b_core // r)
        assert sum(sizes) == b_core

        prev = None
        r0 = 0
        for ti_, rt in enumerate(sizes):
            st = stage_a(r0, rt, ti_ == 0)
            r0 += rt
            if prev is not None:
                stage_b(prev)
            prev = st
        stage_b(prev)
    nc.finalize()
    return nc


_NC_CACHE: dict = {}


def _get_nc(b_core: int, r: int) -> bass.Bass:
    key = (b_core, r)
    if key not in _NC_CACHE:
        _NC_CACHE[key] = build_nc(b_core, r)
    return _NC_CACHE[key]


def _prep_host(W_ff1, b_ff1, W_ff2, b_ff2, W_ta, b_ta, W_tb, b_tb,
               W_in, b_in, input_b, W_r, r_b):
    f32 = lambda a: np.asarray(a, dtype=np.float32)
    W_ff1 = f32(W_ff1); W_ff2 = f32(W_ff2)
    W_tab = f32(W_ta) + f32(W_tb)
    W_in = f32(W_in); W_r = f32(W_r)

    def chunkT(W, m, k):
        return np.ascontiguousarray(W[m * 128:(m + 1) * 128,
                                      k * 128:(k + 1) * 128].T)

    wstack = np.zeros([N_BW, 128, 128], dtype=NP_BF16)
    wstack[BW_COL[("ic", 0, 0)]] = chunkT(W_in, 0, 0).astype(NP_BF16)
    wstack[BW_COL[("ic", 0, 1)]] = chunkT(W_in, 1, 0).astype(NP_BF16)
    for k in range(2):
        for m in range(2):
            wstack[BW_COL[("rc", k, m)]] = chunkT(W_r, m, k).astype(NP_BF16)

    w8stack = np.zeros([N_W8, 2, 128, 128], dtype=NP_FP8)
    for m in range(2):
        # ff1 pair 0 = (x-chunk, hx0-chunk); pair 1 = (hx1-chunk, ZERO)
        w8stack[W8_COL[("ff1", 0, m)], 0] = (WS * chunkT(W_ff1, m, 0)).astype(NP_FP8)
        w8stack[W8_COL[("ff1", 0, m)], 1] = (WS * chunkT(W_ff1, m, 1)).astype(NP_FP8)
        w8stack[W8_COL[("ff1", 1, m)], 0] = (WS * chunkT(W_ff1, m, 2)).astype(NP_FP8)
        # pair 1 slot 1 stays zero (moving slice 3 is x again)
        w8stack[W8_COL[("ff2", 0, m)], 0] = (WS * chunkT(W_ff2, m, 0)).astype(NP_FP8)
        w8stack[W8_COL[("ff2", 0, m)], 1] = (WS * chunkT(W_ff2, m, 1)).astype(NP_FP8)
        w8stack[W8_COL[("tab", 0, m)], 0] = (WS * chunkT(W_tab, m, 0)).astype(NP_FP8)
        w8stack[W8_COL[("tab", 0, m)], 1] = (WS * chunkT(W_tab, m, 1)).astype(NP_FP8)

    biases = {
        "ff1": f32(b_ff1),
        "ff2": f32(b_ff2),
        "tab": 0.5 * (f32(b_ta) + f32(b_tb)),
        "ic": f32(b_in) + f32(input_b),
        "rc": f32(r_b),
    }
    bstack = np.zeros([128, 10], dtype=np.float32)
    for li, layer in enumerate(LAYERS):
        for m in range(2):
            bstack[:, 2 * li + m] = biases[layer][m * 128:(m + 1) * 128]
    return wstack, w8stack, bstack


def _run(inputs: dict, b_core: int = B_CORE, r: int = R, n_cores: int = N_CORES,
         **run_kwargs):
    x = np.asarray(inputs["x"], dtype=np.float32)
    hx = np.asarray(inputs["hx"], dtype=np.float32)
    wstack, w8stack, bstack = _prep_host(
        inputs["W_ff1"], inputs["b_ff1"], inputs["W_ff2"], inputs["b_ff2"],
        inputs["W_ta"], inputs["b_ta"], inputs["W_tb"], inputs["b_tb"],
        inputs["W_in"], inputs["b_in"], inputs["input_b"], inputs["W_r"],
        inputs["r_b"])
    nc = _get_nc(b_core, r)
    in_maps = []
    for c in range(n_cores):
        sl = slice(c * b_core, (c + 1) * b_core)
        in_maps.append({
            "x": np.ascontiguousarray(x[sl]),
            "hx": np.ascontiguousarray(hx[sl]),
            "wstack": wstack,
            "w8stack": w8stack,
            "bstack": bstack,
        })
    res = run_bass_kernel_spmd(nc, in_maps, list(range(n_cores)), **run_kwargs)
    out = np.concatenate([m["out"] for m in res.results], axis=0)
    return out, res


def kernel(**inputs):
    out, _ = _run(inputs)
    return (out, out)


# revision 16
# speedup vs baseline: 1.1338x; 1.1338x over previous
"""Trainium2 Bass kernel for the CfC cell (nn_CfCCell), data-parallel on 8 cores.

Math (per row):
    ff1 = gelu(x_cat @ W_ff1.T + b_ff1)          x_cat = [x, hx]
    ff2 = gelu(ff1 @ W_ff2.T + b_ff2)
    t   = sigmoid(ff2 @ (W_ta+W_tb).T + b_ta+b_tb)      (TS == 1.0)
    ic  = gelu(x @ W_in.T + b_in + input_b)
    rc  = gelu(hx @ W_r.T + r_b)
    out = hx + t * (ic + rc - hx)

Per-core design (batch sharded 8 ways, megatiles of R rows):
  * Activations feature-major ([H, batch]); x/hx transposed on the PE.
  * ff1/ff2/tab matmuls run fp8-e4m3 DoubleRow (weights host-scaled x64,
    un-scaled via the activation instruction's free `scale`); ic/rc stay
    bf16 because their outputs feed the combine directly and fp8 there
    costs ~1e-2 of relative error.
  * One shared PSUM ring (2 slots x 4 banks) serves both matmul z-tiles
    ([128, 2048] f32 -> single N=2048 activation instruction) and the
    transpose staging tiles ([128, 2048] bf16) - ScalarE instruction
    count is the bottleneck, so big ACTs matter.
  * Combine is 3 DVE ops (s=ic+rc; d=s-hxT; f=(u+1)*d via
    scalar_tensor_tensor), and the final "*0.5 + hx" rides the
    transpose-back PSUM->SBUF copy as another scalar_tensor_tensor.
  * DMA uses a contiguous-row mapping ("(p n) f"): partition p holds
    rows p*nb..p*nb+nb-1 of the megatile, so each descriptor moves a
    multi-KB contiguous DRAM run. The batch permutation is consistent
    end-to-end so results land in the right rows.
  * sigmoid(z) = 0.5*tanh(z/2)+0.5 keeps every ScalarE op in the single
    "gelu_and_others" table set (no table reloads).
"""

from contextlib import ExitStack

import ml_dtypes
import numpy as np

import concourse.bacc as bacc
import concourse.bass as bass
import concourse.mybir as mybir
import concourse.tile as tile
from concourse import masks
from concourse.bass_utils import run_bass_kernel_spmd

AF = mybir.ActivationFunctionType
ALU = mybir.AluOpType
DR = mybir.MatmulPerfMode.DoubleRow
BF16 = mybir.dt.bfloat16
FP8 = mybir.dt.float8e4
F32 = mybir.dt.float32
NP_BF16 = ml_dtypes.bfloat16
NP_FP8 = ml_dtypes.float8_e4m3

B, I, H = 131072, 128, 256
N_CORES = 8
B_CORE = B // N_CORES  # 16384
R = 2048               # megatile rows

WS = 64.0              # fp8 weight scale (undone in ACT scale)

# bf16 weight chunks [128, 128]: ff1 (k=0..2), ic (k=0), rc (k=0,1) x2 m
BW_COL = {("ic", 0, 0): 0, ("ic", 0, 1): 1,
          ("rc", 0, 0): 2, ("rc", 0, 1): 3, ("rc", 1, 0): 4, ("rc", 1, 1): 5,
          ("ff1", 0, 0): 6, ("ff1", 0, 1): 7, ("ff1", 1, 0): 8,
          ("ff1", 1, 1): 9, ("ff1", 2, 0): 10, ("ff1", 2, 1): 11}
N_BW = 12
# fp8 DoubleRow chunks [128, 2, 128]: ff2 1 pair x2m, tab 1x2
W8_COL = {("ff2", 0, 0): 0, ("ff2", 0, 1): 1,
          ("tab", 0, 0): 2, ("tab", 0, 1): 3}
N_W8 = 4
LAYERS = ("ff1", "ff2", "tab", "ic", "rc")
BIAS_COL = {(_l, _m): 2 * _i + _m for _i, _l in enumerate(LAYERS) for _m in range(2)}


def build_nc(b_core: int = B_CORE, r: int = R) -> bass.Bass:
    assert b_core % r == 0 and r % 1024 == 0

    nc = bacc.Bacc("TRN2")
    x_d = nc.dram_tensor("x", [b_core, I], F32, kind="ExternalInput")
    hx_d = nc.dram_tensor("hx", [b_core, H], F32, kind="ExternalInput")
    w_d = nc.dram_tensor("wstack", [N_BW, 128, 128], BF16, kind="ExternalInput")
    w8_d = nc.dram_tensor("w8stack", [N_W8, 2, 128, 128], FP8, kind="ExternalInput")
    b_d = nc.dram_tensor("bstack", [128, 10], F32, kind="ExternalInput")
    out_d = nc.dram_tensor("out", [b_core, H], F32, kind="ExternalOutput")

    with tile.TileContext(nc) as tc, ExitStack() as ctx:
        const = ctx.enter_context(tc.tile_pool(name="const", bufs=1))
        w_sb = const.tile([128, N_BW * 128], BF16)
        nc.sync.dma_start(
            w_sb[:].rearrange("p (c f) -> p c f", c=N_BW),
            w_d[:].rearrange("c p f -> p c f"))
        w8_sb = const.tile([128, N_W8 * 2 * 128], FP8)
        nc.sync.dma_start(
            w8_sb[:].rearrange("p (c k f) -> p c k f", c=N_W8, k=2),
            w8_d[:].rearrange("c k p f -> p c k f"))
        b_sb = const.tile([128, 10], F32)
        nc.sync.dma_start(b_sb[:], b_d[:])
        ident = const.tile([128, 128], BF16)
        masks.make_identity(nc, ident[:])

        io = ctx.enter_context(tc.tile_pool(name="io", bufs=2))
        acts = ctx.enter_context(tc.tile_pool(name="acts", bufs=2))
        tmp = ctx.enter_context(tc.tile_pool(name="tmp", bufs=2))
        # one PSUM ring: slots sized for [128, 2048] f32 (4 banks), used both
        # for matmul z-tiles (f32) and transpose staging (bf16)
        ps = ctx.enter_context(tc.tile_pool(name="ps", bufs=2, space="PSUM"))

        # HAM warm-up: dummy PE work while the first loads land
        warm = ps.tile([128, 1024], BF16, tag="ring")
        for i in range(32):
            nc.tensor.transpose(
                warm[:, (i % 8) * 128:(i % 8 + 1) * 128], ident[:], ident[:])

        xd, hxd, outd = x_d[:], hx_d[:], out_d[:]

        def bw(layer, k, m):
            ci = BW_COL[(layer, k, m)]
            return w_sb[:, ci * 128:(ci + 1) * 128]

        def w8(layer, kp, m):
            ci = W8_COL[(layer, kp, m)]
            return w8_sb[:].rearrange(
                "p (c k f) -> p c k f", c=N_W8, k=2)[:, ci]

        def stage_a(r0, rt, first):
            nbt = rt // 128
            # contiguous-row loads: partition p <- rows r0 + p*nbt .. +nbt-1
            x_nat = io.tile([128, nbt * I], BF16, tag="x_nat")
            hx_nat = io.tile([128, nbt * H], BF16, tag="hx_nat")
            x_src = xd[r0:r0 + rt].rearrange("(p n) f -> p n f", p=128)
            hx_src = hxd[r0:r0 + rt].rearrange("(p n) f -> p n f", p=128)
            ng = 4 if first else 2
            g_nb = nbt // ng
            for g in range(ng):
                ns = slice(g * g_nb, (g + 1) * g_nb)
                nc.gpsimd.dma_start(
                    x_nat[:].rearrange("p (n f) -> p n f", n=nbt)[:, ns],
                    x_src[:, ns])
                nc.gpsimd.dma_start(
                    hx_nat[:].rearrange("p (n f) -> p n f", n=nbt)[:, ns],
                    hx_src[:, ns])

            # transpose to feature-major: slices [x | hx0 | hx1], bf16
            xh_T = acts.tile([128, 3 * r], BF16, tag="xh_T")
            npack = 2 if first else 1   # tp tiles per input chunk
            p_nb = nbt // npack

            def tpose(src_tile, src_f, src_c, dsts):
                for q in range(npack):
                    tp = ps.tile([128, p_nb * 128], BF16, tag="ring")
                    for i2 in range(p_nb):
                        n = q * p_nb + i2
                        nc.tensor.transpose(
                            tp[:, i2 * 128:(i2 + 1) * 128],
                            src_tile[:, n * src_f + src_c * 128:
                                     n * src_f + (src_c + 1) * 128],
                            ident[:])
                    cs = q * p_nb * 128
                    for dt_, sl in dsts:
                        nc.vector.tensor_copy(
                            dt_[:, sl * r + cs:sl * r + cs + p_nb * 128],
                            tp[:, 0:p_nb * 128])

            tpose(x_nat, I, 0, [(xh_T, 0)])
            tpose(hx_nat, H, 0, [(xh_T, 1)])
            tpose(hx_nat, H, 1, [(xh_T, 2)])

            def unit_fp8(layer, n_pairs, moving, func, scale, m, out_t, osl):
                mm = ps.tile([128, rt], F32, tag="ring")
                for kp in range(n_pairs):
                    for j in range(rt // 512):
                        sl = slice(j * 512, (j + 1) * 512)
                        nc.tensor.matmul(
                            mm[:, sl], w8(layer, kp, m),
                            moving[:, kp * 2:(kp + 1) * 2, sl],
                            start=(kp == 0), stop=(kp == n_pairs - 1),
                            perf_mode=DR)
                col = BIAS_COL[(layer, m)]
                nc.scalar.activation(
                    out_t[:, osl], mm[:], func,
                    bias=b_sb[:, col:col + 1], scale=scale)

            def unit_bf16(layer, chunks, m, out_t, osl):
                # chunks: list of (weight_k, xh_T slice index)
                mm = ps.tile([128, rt], F32, tag="ring")
                for ki, (k, c) in enumerate(chunks):
                    for j in range(rt // 512):
                        sl = slice(j * 512, (j + 1) * 512)
                        nc.tensor.matmul(
                            mm[:, sl], bw(layer, k, m),
                            xh_T[:, c * r + j * 512:c * r + (j + 1) * 512],
                            start=(ki == 0), stop=(ki == len(chunks) - 1))
                col = BIAS_COL[(layer, m)]
                nc.scalar.activation(
                    out_t[:, osl], mm[:], AF.Gelu,
                    bias=b_sb[:, col:col + 1], scale=1.0)

            def mov4(tile_, nsl):
                # [128, nsl, rt] view over a [128, nsl*r] tile (slice
                # stride r, first rt cols of each slice used)
                return tile_[:].rearrange("p (c b) -> p c b", c=nsl)[:, :, 0:rt]

            ff1 = acts.tile([128, 2 * r], FP8, tag="ff1")
            ff1_m = mov4(ff1, 2)
            ic = acts.tile([128, 2 * r], BF16, tag="ic")
            rc = acts.tile([128, 2 * r], BF16, tag="rc")
            ff2 = acts.tile([128, 2 * r], FP8, tag="ff2")
            ff2_m = mov4(ff2, 2)
            u = acts.tile([128, 2 * r], BF16, tag="u")

            for m in range(2):
                unit_bf16("ff1", [(0, 0), (1, 1), (2, 2)], m,
                          ff1, slice(m * r, m * r + rt))
            for m in range(2):
                unit_bf16("ic", [(0, 0)], m, ic, slice(m * r, m * r + rt))
            for m in range(2):
                unit_bf16("rc", [(0, 1), (1, 2)], m,
                          rc, slice(m * r, m * r + rt))
            for m in range(2):
                unit_fp8("ff2", 1, ff1_m, AF.Gelu, 1.0 / WS, m,
                         ff2, slice(m * r, m * r + rt))
            for m in range(2):
                unit_fp8("tab", 1, ff2_m, AF.Tanh, 0.5 / WS, m,
                         u, slice(m * r, m * r + rt))
            return {"r0": r0, "rt": rt, "xh_T": xh_T, "u": u, "ic": ic,
                    "rc": rc, "hx_nat": hx_nat}

        def stage_b(st):
            r0, rt = st["r0"], st["rt"]
            xh_T, u, ic, rc, hx_nat = (st["xh_T"], st["u"], st["ic"],
                                       st["rc"], st["hx_nat"])
            nbt = rt // 128
            # f = t*(ic+rc-hx), t = 0.5*u+0.5; all 2x-rate bf16 DVE ops
            f = acts.tile([128, 2 * r], BF16, tag="f")
            for m in range(2):
                msl = slice(m * r, m * r + rt)
                s = tmp.tile([128, r], BF16, tag="s")
                nc.vector.tensor_add(s[:, 0:rt], ic[:, msl], rc[:, msl])
                d = tmp.tile([128, r], BF16, tag="d")
                nc.vector.tensor_sub(
                    d[:, 0:rt], s[:, 0:rt],
                    xh_T[:, (1 + m) * r:(1 + m) * r + rt])
                ti = tmp.tile([128, r], BF16, tag="ti")
                nc.vector.tensor_scalar(
                    ti[:, 0:rt], u[:, msl], 0.5, 0.5, ALU.mult, ALU.add)
                nc.vector.tensor_mul(f[:, msl], ti[:, 0:rt], d[:, 0:rt])

            # transpose back in n-pairs (m0,m1), fuse "*0.5 + hx" into the
            # PSUM->SBUF move, store with the same contiguous-row mapping
            out_nat = io.tile([128, nbt * H], BF16, tag="out_nat")
            out_dst = outd[r0:r0 + rt].rearrange("(p n) f -> p n f", p=128)
            nq = 2
            q_nb = nbt // nq
            for q in range(nq):
                tp = ps.tile([128, q_nb * 256], BF16, tag="ring")
                for i2 in range(q_nb):
                    n = q * q_nb + i2
                    for m in range(2):
                        nc.tensor.transpose(
                            tp[:, i2 * 256 + m * 128:i2 * 256 + (m + 1) * 128],
                            f[:, m * r + n * 128:m * r + (n + 1) * 128],
                            ident[:])
                csl = slice(q * q_nb * 256, (q + 1) * q_nb * 256)
                nc.vector.tensor_add(out_nat[:, csl], tp[:], hx_nat[:, csl])
                nc.gpsimd.dma_start(
                    out_dst[:, q * q_nb:(q + 1) * q_nb],
                    out_nat[:, csl].rearrange("p (n f) -> p n f", n=q_nb))

        if b_core > 2 * r:
            sizes = [1024] + [r] * ((b_core - 2048) // r) + [1024]
        else:
            sizes = [r] * (b_core // r)
        assert sum(sizes) == b_core

        prev = None
        r0 = 0
        for ti_, rt in enumerate(sizes):
            st = stage_a(r0, rt, ti_ == 0)
            r0 += rt
            if prev is not None:
                stage_b(prev)
            prev = st
        stage_b(prev)
    nc.finalize()
    return nc


_NC_CACHE: dict = {}


def _get_nc(b_core: int, r: int) -> bass.Bass:
    key = (b_core, r)
    if key not in _NC_CACHE:
        _NC_CACHE[key] = build_nc(b_core, r)
    return _NC_CACHE[key]


def _prep_host(W_ff1, b_ff1, W_ff2, b_ff2, W_ta, b_ta, W_tb, b_tb,
               W_in, b_in, input_b, W_r, r_b):
    f32 = lambda a: np.asarray(a, dtype=np.float32)
    W_ff1 = f32(W_ff1); W_ff2 = f32(W_ff2)
    W_tab = f32(W_ta) + f32(W_tb)
    W_in = f32(W_in); W_r = f32(W_r)

    def chunkT(W, m, k):
        return np.ascontiguousarray(W[m * 128:(m + 1) * 128,
                                      k * 128:(k + 1) * 128].T)

    wstack = np.zeros([N_BW, 128, 128], dtype=NP_BF16)
    wstack[BW_COL[("ic", 0, 0)]] = chunkT(W_in, 0, 0).astype(NP_BF16)
    wstack[BW_COL[("ic", 0, 1)]] = chunkT(W_in, 1, 0).astype(NP_BF16)
    for k in range(2):
        for m in range(2):
            wstack[BW_COL[("rc", k, m)]] = chunkT(W_r, m, k).astype(NP_BF16)
    for k in range(3):
        for m in range(2):
            wstack[BW_COL[("ff1", k, m)]] = chunkT(W_ff1, m, k).astype(NP_BF16)

    w8stack = np.zeros([N_W8, 2, 128, 128], dtype=NP_FP8)
    for m in range(2):
        w8stack[W8_COL[("ff2", 0, m)], 0] = (WS * chunkT(W_ff2, m, 0)).astype(NP_FP8)
        w8stack[W8_COL[("ff2", 0, m)], 1] = (WS * chunkT(W_ff2, m, 1)).astype(NP_FP8)
        w8stack[W8_COL[("tab", 0, m)], 0] = (WS * chunkT(W_tab, m, 0)).astype(NP_FP8)
        w8stack[W8_COL[("tab", 0, m)], 1] = (WS * chunkT(W_tab, m, 1)).astype(NP_FP8)

    biases = {
        "ff1": f32(b_ff1),
        "ff2": f32(b_ff2),
        "tab": 0.5 * (f32(b_ta) + f32(b_tb)),
        "ic": f32(b_in) + f32(input_b),
        "rc": f32(r_b),
    }
    bstack = np.zeros([128, 10], dtype=np.float32)
    for li, layer in enumerate(LAYERS):
        for m in range(2):
            bstack[:, 2 * li + m] = biases[layer][m * 128:(m + 1) * 128]
    return wstack, w8stack, bstack


def _run(inputs: dict, b_core: int = B_CORE, r: int = R, n_cores: int = N_CORES,
         **run_kwargs):
    x = np.asarray(inputs["x"], dtype=np.float32)
    hx = np.asarray(inputs["hx"], dtype=np.float32)
    wstack, w8stack, bstack = _prep_host(
        inputs["W_ff1"], inputs["b_ff1"], inputs["W_ff2"], inputs["b_ff2"],
        inputs["W_ta"], inputs["b_ta"], inputs["W_tb"], inputs["b_tb"],
        inputs["W_in"], inputs["b_in"], inputs["input_b"], inputs["W_r"],
        inputs["r_b"])
    nc = _get_nc(b_core, r)
    in_maps = []
    for c in range(n_cores):
        sl = slice(c * b_core, (c + 1) * b_core)
        in_maps.append({
            "x": np.ascontiguousarray(x[sl]),
            "hx": np.ascontiguousarray(hx[sl]),
            "wstack": wstack,
            "w8stack": w8stack,
            "bstack": bstack,
        })
    res = run_bass_kernel_spmd(nc, in_maps, list(range(n_cores)), **run_kwargs)
    out = np.concatenate([m["out"] for m in res.results], axis=0)
    return out, res


def kernel(**inputs):
    out, _ = _run(inputs)
    return (out, out)
